# revision 39
# baseline (speedup 1.0000x reference)
"""Trainium2 Bass kernel for nn_B_NNs_34789235097695.

Problem: per batch element b (B=262144):
    y   = MLP(s_Ddot[b])  (3 -> 128 -> 128 -> 128 -> 3, tanh, fp32)
    K   = diag geometry from (q[b], s[b])
    A   = 3x3 geometry matrix from (q[b], s[b])
    out = Kdiag * solve(A, y)             -> [B, 3, 1]

Strategy (8 cores, pure data parallel, 32768 batch rows per core):
  - ScalarE (ACT) is the floor: 3 tanh layers x 32768 columns/core at
    ~1.1us per [128,1024] tanh (HW-measured; fp8 output writes shave
    ~17% vs f16). Everything else hides under a continuous tanh stream.
  - MLP on PE in "hidden-on-partitions" layout (psum = W^T @ xT), f16
    layer-0 / fp8e4 hidden-layer operand chain, chunks of 1024 batch
    columns, 3-stage skewed software pipeline across chunks (ACT FIFO
    runs T0(i), T1(i-1), T2(i-2)). fp8 halves PE operand power, which
    keeps the HAM power manager from duty-cycling the PE to half speed
    mid-stream (measured: throttle windows shrink from ~45% to ~15% of
    the run and tanh slots stay dense).
  - b0 is folded into W0 via a host-side ones-row on the transposed input
    (contract dim 3 -> 4), removing the b0 DMA from the first-tanh
    critical path.
  - Host-side batch permutation pi(u) = 256*(u&127) + (u>>7): the MLP
    stream processes batch in an order such that each staging group's
    layer-3 output lands in consecutive *f-columns* of the
    batch-on-partitions geometry layout (b = 256*p + f). The final
    combine (out = Kdiag/det * adj(A) @ (y+b3)) then runs incrementally
    per group, hidden under the tanh stream, instead of as a serial tail
    after the last chunk. q/s/out DMAs keep their cheap contiguous
    per-partition runs; only s_DdotT is permuted (free, host-side).
  - Geometry (sin/cos polynomials, Kdiag, A) on VectorE; cofactors split
    GpSimd/DVE so adj(A) is ready before the first group combine; det
    and reciprocal on DVE to avoid cross-engine head-of-line blocks.
  - Layer 3 as 8 stationary-h3 matmuls per chunk (lhsT = h3 128-column
    block, rhs = W3 [128, 3]): each [128, 3] output lands directly in
    batch-on-partitions order (with pi, block m of chunk ci IS yB3
    f-column 8*ci+m), so one tiny [128, 24] DVE copy per chunk replaces
    the [3, 1024] PSUM drain + staging tile + respread DMAs entirely.
  - Per-group combine: 6 wide DVE ops on [128, 3, nf] slices using
    stride-0 broadcast APs over a consolidated cofactor tile C_all,
    folding the b3 bias add into scalar_tensor_tensor; gated only on
    same-queue DVE copies (no DMA semaphore hops). Per-group output DMA
    overlaps the stream. The last chunk is its own 8-column mini group
    with split edge tanhs, leaving ~6.5us of combine+DMA+barrier after
    the final tanh.

Self-contained: hardcodes all shapes; needs only the container's Bass
runtime (/opt/trn_rl_repo or the axon site copy) and the NeuronCores.
"""

import sys

for _p in ("/opt/trn_rl_repo", "/root/.axon_site/_ro/trn_rl_repo"):
    if _p not in sys.path:
        sys.path.append(_p)

import numpy as np

B_FULL = 262144
N_CORES = 8
BC = B_FULL // N_CORES          # 32768 batch rows per core
F = BC // 128                   # 256 free columns in geometry layout
H = 128

RB = 0.06                       # BASE_RADIUS
RE = 0.045                      # END_EFFECTOR_RADIUS
LA = 0.176                      # LOWER_ARM_LENGTH

MM_DTYPE = "f16"                # layer-0 operand dtype ("f16" | "f8e4")
H_DTYPE = "f8e4"                # hidden-layer operand dtype ("f16" | "f8e4")

_alpha = np.deg2rad(np.array([-30.0, 90.0, 210.0], np.float32))
CA = [float(v) for v in np.cos(_alpha)]
SA = [float(v) for v in np.sin(_alpha)]

# sin (odd, t=x^2): c1..c9 ; cos (even): d0..d5   -- for q in [0, 1)
_SC = [1.0, -1.0 / 6, 1.0 / 120, -1.0 / 5040, 1.0 / 362880]
_CC = [1.0, -0.5, 1.0 / 24, -1.0 / 720, 1.0 / 40320, -1.0 / 3628800]


CHUNK = 1024                    # 2 PSUM banks per stage tile
GRP = 4                         # chunks per group / 32 f-columns
GF = GRP * CHUNK // 128         # f-columns per group (32)


def _chunks():
    assert BC % CHUNK == 0
    return [(i * CHUNK, CHUNK) for i in range(BC // CHUNK)]


def _emit(nc, tc, ctx):
    import concourse.bass as bass
    from concourse import mybir

    f32 = mybir.dt.float32
    ALU = mybir.AluOpType
    ACTF = mybir.ActivationFunctionType

    fmm = {
        "f16": mybir.dt.float16,
        "f8e4": mybir.dt.float8e4,
    }[MM_DTYPE]
    fh = {
        "f16": mybir.dt.float16,
        "f8e4": mybir.dt.float8e4,
    }[H_DTYPE]

    # ---------------- DRAM tensors (per-core shapes) ----------------
    q_d = nc.dram_tensor("q", [BC, 3], f32, kind="ExternalInput").ap()
    s_d = nc.dram_tensor("s", [BC, 3], f32, kind="ExternalInput").ap()
    # permuted + ones-row-augmented transposed MLP input (host-side)
    sddT_d = nc.dram_tensor("sddT", [4, BC], fmm, kind="ExternalInput").ap()
    W_d = [
        nc.dram_tensor("W0", [4, H], fmm, kind="ExternalInput").ap(),  # b0 folded
        nc.dram_tensor("W1", [H, H], fh, kind="ExternalInput").ap(),
        nc.dram_tensor("W2", [H, H], fh, kind="ExternalInput").ap(),
        nc.dram_tensor("W3", [H, 3], fh, kind="ExternalInput").ap(),
    ]
    b_d = {
        1: nc.dram_tensor("b1", [H], f32, kind="ExternalInput").ap(),
        2: nc.dram_tensor("b2", [H], f32, kind="ExternalInput").ap(),
        3: nc.dram_tensor("b3", [3], f32, kind="ExternalInput").ap(),
    }
    out_d = nc.dram_tensor("out", [BC, 3], f32, kind="ExternalOutput").ap()
    outv = out_d.rearrange("(p f) c -> p f c", p=128)

    # ---------------- pools ----------------
    singles = ctx.enter_context(tc.tile_pool(name="singles", bufs=1))
    geo = ctx.enter_context(tc.tile_pool(name="geo", bufs=1))
    pool_in = ctx.enter_context(tc.tile_pool(name="pool_in", bufs=4))
    pool_h = ctx.enter_context(tc.tile_pool(name="pool_h", bufs=12))
    pool_cmb = ctx.enter_context(tc.tile_pool(name="pool_cmb", bufs=2))
    # 3 stage tags (mm0/mm1/mm2) x 1 buf x 2 banks + l3 [128,24] x 2 = 8
    psum_mm = ctx.enter_context(tc.tile_pool(name="psum_mm", bufs=1, space="PSUM"))
    psum_l3 = ctx.enter_context(tc.tile_pool(name="psum_l3", bufs=2, space="PSUM"))

    # ---------------- prologue warm-up ----------------
    # A couple of tiny matmuls prime the PE pipeline; the dummy activation
    # pulls the ~1.3us tanh ACT_TABLE_LOAD (and the const-bias TENSOR_LOAD)
    # into the input-DMA prologue. A long burst here would delay mm0(0) on
    # the in-order PE queue and add power that invites HAM throttling.
    # ~12 x 128-row dummy matmuls ≈ 1.3us: keeps the PE out of its cold
    # pstate until just before the first input chunk lands (~2.0us after
    # the queue opens) WITHOUT blocking mm0(0) behind them — the PE queue
    # is issue-in-order, so a longer burst would delay the first chunk.
    warm = singles.tile([128, 128], fmm, name="warm", tag="warm")
    nc.vector.memset(warm, 0.0)
    wpsum = psum_mm.tile([128, CHUNK], f32, name="wpsum", tag="mm0")
    for _ in range(12):
        nc.tensor.matmul(wpsum[0:3, 0:128], warm[:, 0:3], warm,
                         start=True, stop=True)
    wact = singles.tile([128, 1], f32, name="wact", tag="wact")
    nc.scalar.activation(wact, warm[:, 0:1], ACTF.Tanh)

    # ---------------- constants / weights in SBUF ----------------
    w_sb = [None] * 4
    b_sb = {}

    def load_w(i):
        w = singles.tile(list(W_d[i].shape), W_d[i].dtype, name=f"w{i}sb",
                         tag=f"w{i}sb")
        nc.sync.dma_start(out=w, in_=W_d[i])
        w_sb[i] = w

    def load_b(i):
        # GpSimd SWDGE queue: parallel to the Sync HWDGE chain.
        b = singles.tile([H, 1], f32, name=f"b{i}sb", tag=f"b{i}sb")
        nc.gpsimd.dma_start(out=b, in_=b_d[i].rearrange("(p one) -> p one", one=1))
        b_sb[i] = b

    # b3 broadcast to all partitions: [128, 3]
    b3bc = singles.tile([128, 3], f32, name="b3bc", tag="b3bc")

    def load_rest():
        for i in (1, 2, 3):
            load_w(i)
            if i < 3:
                load_b(i)
        nc.gpsimd.dma_start(
            out=b3bc,
            in_=bass.AP(tensor=b_d[3].tensor, offset=0, ap=[[0, 128], [1, 3]]),
        )

    # interleaved q/s in order B: partition p holds rows [p*F, (p+1)*F).
    # GpSimd SWDGE queue, deferred into the geometry stream: keeps the
    # Sync HWDGE ring free for the latency-critical early sdd chunk loads.
    iq = singles.tile([128, F, 3], f32, name="iq", tag="iq")
    is_ = singles.tile([128, F, 3], f32, name="is_", tag="is_")

    # MLP output, batch-on-partitions order B: yB3[p, c, f] = y[256p+f, c]
    yB3 = singles.tile([128, 3, F], f32, name="yB3", tag="yB3")
    # consolidated cofactors: C_all[p, 3*j+i, f] = C[j][i] at batch 256p+f
    C_all = singles.tile([128, 9, F], f32, name="C_all", tag="C_all")
    # Kdiag/det, per component: Krd3[p, c, f]
    Krd3 = singles.tile([128, 3, F], f32, name="Krd3", tag="Krd3")
    out_int = singles.tile([128, F, 3], f32, name="out_int", tag="out_int")

    # ---------------- geometry op list (drained between chunks) ----------
    G = {}  # name -> AP

    def gt(name, tag=None):
        t = geo.tile([128, F], f32, name=name, tag=tag or name)
        G[name] = t
        return t

    geo_ops = []

    def deferred(fn):
        geo_ops.append(fn)

    vec = nc.vector

    def op_load_iq():
        nc.gpsimd.dma_start(out=iq, in_=q_d.rearrange("(p f) c -> p f c", p=128))

    def op_load_is():
        nc.gpsimd.dma_start(out=is_, in_=s_d.rearrange("(p f) c -> p f c", p=128))

    deferred(op_load_iq)
    deferred(op_load_is)

    # --- phase 1: t = q^2 and cos(q) (cos gates A which gates cofactors)
    def emit_tcos(c):
        x = iq[:, :, c]

        def op_t():
            t = gt(f"t{c}")
            vec.tensor_mul(t, x, x)
        deferred(op_t)

        def op_cos():
            t = G[f"t{c}"]
            d0, d1, d2, d3, d4, d5 = _CC
            w = gt(f"cw{c}", tag="cw")
            vec.scalar_tensor_tensor(w, t, d4 / d5, t, op0=ALU.add, op1=ALU.mult)
            vec.scalar_tensor_tensor(w, w, d3 / d5, t, op0=ALU.add, op1=ALU.mult)
            vec.scalar_tensor_tensor(w, w, d2 / d5, t, op0=ALU.add, op1=ALU.mult)
            vec.scalar_tensor_tensor(w, w, d1 / d5, t, op0=ALU.add, op1=ALU.mult)
            cq = gt(f"cq{c}")
            vec.tensor_scalar(cq, w, d5, 1.0, op0=ALU.mult, op1=ALU.add)
        deferred(op_cos)

    # --- phase 2: A rows (row index = coordinate, col index = arm c)
    def emit_a(c):
        s0, s1, s2 = is_[:, :, 0], is_[:, :, 1], is_[:, :, 2]

        def op_a():
            cq = G[f"cq{c}"]
            dR = RE - RB
            a0 = gt(f"a0{c}")
            vec.tensor_scalar(a0, cq, -LA * CA[c], dR * CA[c],
                              op0=ALU.mult, op1=ALU.add)
            vec.tensor_add(a0, a0, s0)
            a1 = gt(f"a1{c}")
            vec.tensor_scalar(a1, cq, -LA * SA[c], dR * SA[c],
                              op0=ALU.mult, op1=ALU.add)
            vec.tensor_add(a1, a1, s1)
            a2 = gt(f"a2{c}")
            vec.scalar_tensor_tensor(a2, cq, -LA, s2, op0=ALU.mult, op1=ALU.add)
        deferred(op_a)

    # --- phase 3a: cofactors C[i][j] of entry (i,j); x_i = sum_j C[j][i]*r_j
    COF = [
        ((0, 0), (1, 1), (2, 2), (1, 2), (2, 1)),
        ((0, 1), (1, 2), (2, 0), (1, 0), (2, 2)),
        ((0, 2), (1, 0), (2, 1), (1, 1), (2, 0)),
        ((1, 0), (0, 2), (2, 1), (0, 1), (2, 2)),
        ((1, 1), (0, 0), (2, 2), (0, 2), (2, 0)),
        ((1, 2), (0, 1), (2, 0), (0, 0), (2, 1)),
        ((2, 0), (0, 1), (1, 2), (0, 2), (1, 1)),
        ((2, 1), (0, 2), (1, 0), (0, 0), (1, 2)),
        ((2, 2), (0, 0), (1, 1), (0, 1), (1, 0)),
    ]

    def emit_cof(spec, eng_name):
        (ci, cj), (pi, pj), (pk, pl), (ni, nj), (nk, nl) = spec

        def op():
            eng = getattr(nc, eng_name)
            m1 = gt(f"cm1_{eng_name}_{ci}{cj}", tag=f"cm1_{eng_name}")
            eng.tensor_mul(m1, G[f"a{pi}{pj}"], G[f"a{pk}{pl}"])
            m2 = gt(f"cm2_{eng_name}_{ci}{cj}", tag=f"cm2_{eng_name}")
            eng.tensor_mul(m2, G[f"a{ni}{nj}"], G[f"a{nk}{nl}"])
            eng.tensor_sub(C_all[:, 3 * ci + cj, :], m1, m2)
        deferred(op)

    # --- phase 3b: sin(q) and Kdiag (needed only by Krd, not by cofactors)
    def emit_sin(c):
        x = iq[:, :, c]

        def op_sin():
            t = G[f"t{c}"]
            c1, c3, c5, c7, c9 = _SC
            w = gt(f"sw{c}", tag="sw")
            vec.scalar_tensor_tensor(w, t, c7 / c9, t, op0=ALU.add, op1=ALU.mult)
            vec.scalar_tensor_tensor(w, w, c5 / c9, t, op0=ALU.add, op1=ALU.mult)
            vec.scalar_tensor_tensor(w, w, c3 / c9, t, op0=ALU.add, op1=ALU.mult)
            vec.tensor_scalar(w, w, c9, 1.0, op0=ALU.mult, op1=ALU.add)
            sq = gt(f"sq{c}")
            vec.tensor_mul(sq, w, x)
        deferred(op_sin)

    def emit_k(c):
        s0, s1, s2 = is_[:, :, 0], is_[:, :, 1], is_[:, :, 2]

        def op_k():
            sq, cq = G[f"sq{c}"], G[f"cq{c}"]
            u = gt(f"ku{c}", tag="ku")
            vec.tensor_scalar(u, s0, CA[c], RB - RE, op0=ALU.mult, op1=ALU.add)
            vec.scalar_tensor_tensor(u, s1, SA[c], u, op0=ALU.mult, op1=ALU.add)
            vec.tensor_mul(u, u, sq)
            w = gt(f"kw{c}", tag="kw")
            vec.tensor_mul(w, s2, cq)
            k = gt(f"K{c}")
            vec.tensor_sub(k, u, w)
        deferred(op_k)

    def op_det():
        # det on DVE from the 3 DVE-computed cofactors: no cross-engine
        # wait at the DVE queue head.
        m1 = gt("dm1")
        vec.tensor_mul(m1, G["a00"], C_all[:, 0, :])
        m2 = gt("dm2")
        vec.tensor_mul(m2, G["a01"], C_all[:, 1, :])
        vec.tensor_add(m1, m1, m2)
        vec.tensor_mul(m2, G["a02"], C_all[:, 2, :])
        det = gt("det")
        vec.tensor_add(det, m1, m2)

    def op_rdet():
        rdet = gt("rdet")
        vec.reciprocal(rdet, G["det"])
        for c in range(3):
            vec.tensor_mul(Krd3[:, c, :], G[f"K{c}"], rdet)

    for c in range(3):
        emit_tcos(c)
    for c in range(3):
        emit_a(c)
    # row 0 cofactors (used by det) on DVE first; split the rest so GpSimd
    # finishes adj(A) before the first group combine needs it.
    for idx, spec in enumerate(COF):
        if idx < 3:
            emit_cof(spec, "vector")
        elif idx in (3, 4):
            emit_cof(spec, "vector")
        else:
            emit_cof(spec, "gpsimd")
    deferred(op_det)
    for c in range(3):
        emit_sin(c)
    for c in range(3):
        emit_k(c)
    deferred(op_rdet)

    # ---------------- chunk groups ---------------------------------------
    # (first_chunk, n_chunks) per staging group; the last chunk gets its
    # own mini group so only an 8-column respread+combine+store remains
    # after the final tanh.
    GROUPS = [(4 * g, 4) for g in range(7)] + [(28, 3), (31, 1)]
    CHUNK_GROUP = {}
    for gi, (c0, nch) in enumerate(GROUPS):
        for c in range(c0, c0 + nch):
            CHUNK_GROUP[c] = gi

    def group_fslice(gi):
        c0, nch = GROUPS[gi]
        return 8 * c0, 8 * nch

    # ---------------- per-group combine --------------------------------
    # out[:, :, i] = Krd_i * sum_j C[j][i] * (y_j + b3_j), on the group's
    # f-slice as soon as its yB3 columns land.
    def emit_combine(g):
        s0, nf = group_fslice(g)
        sl = slice(s0, s0 + nf)
        shp = [128, 3, nf]
        m = pool_cmb.tile(shp, f32, name=f"cmb_m{g}", tag="cmb_m")
        t = pool_cmb.tile(shp, f32, name=f"cmb_t{g}", tag="cmb_t")

        def ybr(j):
            return yB3[:, j:j + 1, sl].broadcast_to(shp)

        vec.scalar_tensor_tensor(m, ybr(0), b3bc[:, 0:1], C_all[:, 0:3, sl],
                                 op0=ALU.add, op1=ALU.mult)
        vec.scalar_tensor_tensor(t, ybr(1), b3bc[:, 1:2], C_all[:, 3:6, sl],
                                 op0=ALU.add, op1=ALU.mult)
        vec.tensor_add(m, m, t)
        vec.scalar_tensor_tensor(t, ybr(2), b3bc[:, 2:3], C_all[:, 6:9, sl],
                                 op0=ALU.add, op1=ALU.mult)
        vec.tensor_add(m, m, t)
        vec.tensor_mul(out_int[:, sl, :].transpose([0, 2, 1]), m,
                       Krd3[:, :, sl])
        nc.sync.dma_start(out=outv[:, sl, :], in_=out_int[:, sl, :])

    # ---------------- MLP chunks: 3-stage skewed software pipeline -------
    chunks = _chunks()
    n_chunks = len(chunks)
    n_iters = n_chunks + 2
    per_gap = 8

    PS = {}   # (stage, chunk) -> psum tile
    HT = {}   # (stage, chunk) -> h tile

    def st_dma(ci):
        off, S = chunks[ci]
        sddc = pool_in.tile([4, S], fmm, name=f"sdd_{ci}", tag="sdd")
        if ci == 0:
            # two half-loads: mm0(0) k=0 starts on the first half while the
            # second is still in flight (pipeline ramp-in)
            hS = S // 2
            for k in range(2):
                nc.sync.dma_start(out=sddc[:, hS * k:hS * (k + 1)],
                                  in_=sddT_d[:, off + hS * k:off + hS * (k + 1)])
        else:
            nc.sync.dma_start(out=sddc, in_=sddT_d[:, off:off + S])
        HT[("x", ci)] = sddc

    def st_mm(layer, ci):
        _, S = chunks[ci]
        nS = S // 512
        src = HT[("x", ci)] if layer == 0 else HT[(layer - 1, ci)]
        ps = psum_mm.tile([128, S], f32, name=f"ps{layer}_{ci}",
                          tag=f"mm{layer}")
        for k in range(nS):
            nc.tensor.matmul(ps[:, 512 * k:512 * (k + 1)], w_sb[layer],
                             src[:, 512 * k:512 * (k + 1)],
                             start=True, stop=True)
        PS[(layer, ci)] = ps

    # pipeline-edge tanhs are split in half so the consumer of the first
    # half starts one half-tanh earlier (stream ramp-in / tail ramp-out)
    SPLIT_TANH = {(0, 0), (2, n_chunks - 1)}

    def st_tanh(layer, ci):
        _, S = chunks[ci]
        h = pool_h.tile([128, S], fh, name=f"h{layer}_{ci}", tag="h")
        bias = 0.0 if layer == 0 else b_sb[layer]
        if (layer, ci) in SPLIT_TANH:
            hS = S // 2
            for k in range(2):
                sl = slice(hS * k, hS * (k + 1))
                nc.scalar.activation(h[:, sl], PS[(layer, ci)][:, sl],
                                     ACTF.Tanh, bias=bias)
        else:
            nc.scalar.activation(h, PS[(layer, ci)], ACTF.Tanh, bias=bias)
        HT[(layer, ci)] = h
        del PS[(layer, ci)]

    def st_l3(ci):
        # layer 3 as 8 "stationary-h3" matmuls: lhsT = a 128-column block
        # of h3, rhs = W3 [128, 3], so each [128, 3] output lands directly
        # in batch-on-partitions order — with the pi permutation, block m
        # of chunk ci IS yB3 f-column 8*ci + m. One tiny [128, 24] DVE
        # copy per chunk replaces the [3, 1024] drain + staging + respread
        # DMAs of the column-major layout.
        h3 = HT[(2, ci)]
        psb = psum_l3.tile([128, 24], f32, name=f"l3_{ci}", tag="l3")
        for m in range(8):
            nc.tensor.matmul(psb[:, 3 * m:3 * (m + 1)],
                             h3[:, 128 * m:128 * (m + 1)], w_sb[3],
                             start=True, stop=True)
        vec.tensor_copy(
            yB3[:, :, 8 * ci:8 * ci + 8].transpose([0, 2, 1]),
            psb.rearrange("p (m c) -> p m c", c=3),
        )

    # combine(g) is emitted two chunks into the next group: the group's
    # yB3 copies (same DVE queue) and the geometry it reads are complete
    # by then, so the strict-FIFO DVE queue never head-of-line blocks on
    # it. Group 7 (chunks 28-30) combines right after chunk 30's copy;
    # group 8 (final chunk) right after the last copy.
    combine_at = {4 * g + 5: g for g in range(7)}
    combine_at[30] = 7

    load_w(0)
    st_dma(0)
    st_dma(1)
    load_rest()
    st_mm(0, 0)
    for i in range(n_iters):
        if i + 2 < n_chunks:
            st_dma(i + 2)
        if i + 1 < n_chunks:
            st_mm(0, i + 1)
        if i < n_chunks:
            st_tanh(0, i)
            st_mm(1, i)
        if 0 <= i - 1 < n_chunks:
            st_tanh(1, i - 1)
            st_mm(2, i - 1)
        if 0 <= i - 2 < n_chunks:
            st_tanh(2, i - 2)
            st_l3(i - 2)
            if (i - 2) in combine_at:
                emit_combine(combine_at[i - 2])
        for _ in range(per_gap):
            if geo_ops:
                geo_ops.pop(0)()

    while geo_ops:
        geo_ops.pop(0)()
    emit_combine(8)


def build():
    """Build the per-core Bass program (same program for all 8 cores)."""
    from contextlib import ExitStack

    import concourse.bacc as bacc
    import concourse.tile as tile

    nc = bacc.Bacc(trn_type="TRN2", target_bir_lowering=False, debug=False)
    with tile.TileContext(nc) as tc:
        with ExitStack() as ctx:
            _emit(nc, tc, ctx)
    nc.compile()
    return nc


_NC_CACHE = []


def _shard_inputs(inputs):
    f32 = np.float32
    import ml_dtypes

    fmm = {"f16": np.float16, "f8e4": ml_dtypes.float8_e4m3}[MM_DTYPE]
    fhn = {"f16": np.float16, "f8e4": ml_dtypes.float8_e4m3}[H_DTYPE]
    q = np.ascontiguousarray(np.asarray(inputs["q"], dtype=f32))
    s = np.ascontiguousarray(np.asarray(inputs["s"], dtype=f32))
    sdd = np.asarray(inputs["s_Ddot"], dtype=f32)
    weights = {}
    W0b0 = np.concatenate(
        [np.asarray(inputs["W0"], dtype=f32),
         np.asarray(inputs["b0"], dtype=f32)[None, :]], axis=0)
    weights["W0"] = np.ascontiguousarray(W0b0.astype(fmm))
    for k in ("W1", "W2", "W3"):
        weights[k] = np.ascontiguousarray(np.asarray(inputs[k], dtype=f32).astype(fhn))
    for k in ("b1", "b2", "b3"):
        weights[k] = np.ascontiguousarray(np.asarray(inputs[k], dtype=f32))
    ones = np.ones((1, BC), dtype=fmm)
    in_maps = []
    for c in range(N_CORES):
        sl = slice(c * BC, (c + 1) * BC)
        # pi-permuted transpose: column u = 128f+p holds batch row 256p+f
        X = sdd[sl].reshape(128, F, 3).transpose(2, 1, 0).reshape(3, BC)
        m = {
            "q": q[sl],
            "s": s[sl],
            "sddT": np.ascontiguousarray(
                np.concatenate([X.astype(fmm), ones], axis=0)),
        }
        m.update(weights)
        in_maps.append(m)
    return in_maps


def kernel(**inputs) -> np.ndarray:
    from concourse import bass_utils

    if not _NC_CACHE:
        _NC_CACHE.append(build())
    nc = _NC_CACHE[0]

    in_maps = _shard_inputs(inputs)
    # The axon relay occasionally fails a first execution with
    # NRT_EXEC_UNIT_UNRECOVERABLE; an immediate retry succeeds.
    last_err = None
    for _attempt in range(3):
        try:
            res = bass_utils.run_bass_kernel_spmd(
                nc, in_maps, core_ids=list(range(N_CORES)))
            break
        except Exception as e:  # jax.errors.JaxRuntimeError (transient)
            last_err = e
    else:
        raise last_err
    out = np.concatenate([res.results[c]["out"] for c in range(N_CORES)], axis=0)
    return out.reshape(B_FULL, 3, 1).astype(np.float32)


if __name__ == "__main__":
    nc = build()
    print("built OK")


# revision 41
# speedup vs baseline: 1.0331x; 1.0331x over previous
"""Trainium2 Bass kernel for nn_B_NNs_34789235097695.

Problem: per batch element b (B=262144):
    y   = MLP(s_Ddot[b])  (3 -> 128 -> 128 -> 128 -> 3, tanh, fp32)
    K   = diag geometry from (q[b], s[b])
    A   = 3x3 geometry matrix from (q[b], s[b])
    out = Kdiag * solve(A, y)             -> [B, 3, 1]

Strategy (8 cores, pure data parallel, 32768 batch rows per core):
  - ScalarE (ACT) is the floor: 3 tanh layers x 32768 columns/core at
    ~1.1us per [128,1024] tanh (HW-measured; fp8 output writes shave
    ~17% vs f16). Everything else hides under a continuous tanh stream.
  - MLP on PE in "hidden-on-partitions" layout (psum = W^T @ xT), f16
    layer-0 / fp8e4 hidden-layer operand chain, chunks of 1024 batch
    columns, 3-stage skewed software pipeline across chunks (ACT FIFO
    runs T0(i), T1(i-1), T2(i-2)). fp8 halves PE operand power, which
    keeps the HAM power manager from duty-cycling the PE to half speed
    mid-stream (measured: throttle windows shrink from ~45% to ~15% of
    the run and tanh slots stay dense).
  - b0 is folded into W0 via a host-side ones-row on the transposed input
    (contract dim 3 -> 4), removing the b0 DMA from the first-tanh
    critical path.
  - Host-side batch permutation pi(u) = 256*(u&127) + (u>>7): the MLP
    stream processes batch in an order such that each staging group's
    layer-3 output lands in consecutive *f-columns* of the
    batch-on-partitions geometry layout (b = 256*p + f). The final
    combine (out = Kdiag/det * adj(A) @ (y+b3)) then runs incrementally
    per group, hidden under the tanh stream, instead of as a serial tail
    after the last chunk. q/s/out DMAs keep their cheap contiguous
    per-partition runs; only s_DdotT is permuted (free, host-side).
  - Geometry (sin/cos polynomials, Kdiag, A) on VectorE; cofactors split
    GpSimd/DVE so adj(A) is ready before the first group combine; det
    and reciprocal on DVE to avoid cross-engine head-of-line blocks.
  - Layer 3 as 8 stationary-h3 matmuls per chunk (lhsT = h3 128-column
    block, rhs = W3 [128, 3]): each [128, 3] output lands directly in
    batch-on-partitions order (with pi, block m of chunk ci IS yB3
    f-column 8*ci+m), so one tiny [128, 24] DVE copy per chunk replaces
    the [3, 1024] PSUM drain + staging tile + respread DMAs entirely.
  - Per-group combine: 6 wide DVE ops on [128, 3, nf] slices using
    stride-0 broadcast APs over a consolidated cofactor tile C_all,
    folding the b3 bias add into scalar_tensor_tensor; gated only on
    same-queue DVE copies (no DMA semaphore hops). Per-group output DMA
    overlaps the stream. The last chunk is its own 8-column mini group
    with split edge tanhs, leaving ~6.5us of combine+DMA+barrier after
    the final tanh.

Self-contained: hardcodes all shapes; needs only the container's Bass
runtime (/opt/trn_rl_repo or the axon site copy) and the NeuronCores.
"""

import sys

for _p in ("/opt/trn_rl_repo", "/root/.axon_site/_ro/trn_rl_repo"):
    if _p not in sys.path:
        sys.path.append(_p)

import numpy as np

B_FULL = 262144
N_CORES = 8
BC = B_FULL // N_CORES          # 32768 batch rows per core
F = BC // 128                   # 256 free columns in geometry layout
H = 128

RB = 0.06                       # BASE_RADIUS
RE = 0.045                      # END_EFFECTOR_RADIUS
LA = 0.176                      # LOWER_ARM_LENGTH

MM_DTYPE = "f16"                # layer-0 operand dtype ("f16" | "f8e4")
H_DTYPE = "f8e4"                # hidden-layer operand dtype ("f16" | "f8e4")

_alpha = np.deg2rad(np.array([-30.0, 90.0, 210.0], np.float32))
CA = [float(v) for v in np.cos(_alpha)]
SA = [float(v) for v in np.sin(_alpha)]

# sin (odd, t=x^2): c1..c9 ; cos (even): d0..d5   -- for q in [0, 1)
_SC = [1.0, -1.0 / 6, 1.0 / 120, -1.0 / 5040, 1.0 / 362880]
_CC = [1.0, -0.5, 1.0 / 24, -1.0 / 720, 1.0 / 40320, -1.0 / 3628800]


CHUNK = 1024                    # 2 PSUM banks per stage tile
GRP = 4                         # chunks per group / 32 f-columns
GF = GRP * CHUNK // 128         # f-columns per group (32)


def _chunks():
    assert BC % CHUNK == 0
    return [(i * CHUNK, CHUNK) for i in range(BC // CHUNK)]


def _emit(nc, tc, ctx):
    import concourse.bass as bass
    from concourse import mybir

    f32 = mybir.dt.float32
    ALU = mybir.AluOpType
    ACTF = mybir.ActivationFunctionType

    fmm = {
        "f16": mybir.dt.float16,
        "f8e4": mybir.dt.float8e4,
    }[MM_DTYPE]
    fh = {
        "f16": mybir.dt.float16,
        "f8e4": mybir.dt.float8e4,
    }[H_DTYPE]

    # ---------------- DRAM tensors (per-core shapes) ----------------
    q_d = nc.dram_tensor("q", [BC, 3], f32, kind="ExternalInput").ap()
    s_d = nc.dram_tensor("s", [BC, 3], f32, kind="ExternalInput").ap()
    # permuted + ones-row-augmented transposed MLP input (host-side)
    sddT_d = nc.dram_tensor("sddT", [4, BC], fmm, kind="ExternalInput").ap()
    W_d = [
        nc.dram_tensor("W0", [4, H], fmm, kind="ExternalInput").ap(),  # b0 folded
        nc.dram_tensor("W1", [H, H], fh, kind="ExternalInput").ap(),
        nc.dram_tensor("W2", [H, H], fh, kind="ExternalInput").ap(),
        nc.dram_tensor("W3", [H, 3], fh, kind="ExternalInput").ap(),
    ]
    b_d = {
        1: nc.dram_tensor("b1", [H], f32, kind="ExternalInput").ap(),
        2: nc.dram_tensor("b2", [H], f32, kind="ExternalInput").ap(),
        3: nc.dram_tensor("b3", [3], f32, kind="ExternalInput").ap(),
    }
    out_d = nc.dram_tensor("out", [BC, 3], f32, kind="ExternalOutput").ap()
    outv = out_d.rearrange("(p f) c -> p f c", p=128)

    # ---------------- pools ----------------
    singles = ctx.enter_context(tc.tile_pool(name="singles", bufs=1))
    geo = ctx.enter_context(tc.tile_pool(name="geo", bufs=1))
    pool_in = ctx.enter_context(tc.tile_pool(name="pool_in", bufs=4))
    pool_h = ctx.enter_context(tc.tile_pool(name="pool_h", bufs=12))
    pool_cmb = ctx.enter_context(tc.tile_pool(name="pool_cmb", bufs=2))
    pool_stg = ctx.enter_context(tc.tile_pool(name="pool_stg", bufs=3))
    # 3 stage tags (mm0/mm1/mm2) x 1 buf x 2 banks + l3 [3,1024] 2 banks = 8
    psum_mm = ctx.enter_context(tc.tile_pool(name="psum_mm", bufs=1, space="PSUM"))
    psum_l3 = ctx.enter_context(tc.tile_pool(name="psum_l3", bufs=1, space="PSUM"))

    # ---------------- prologue warm-up ----------------
    # A couple of tiny matmuls prime the PE pipeline; the dummy activation
    # pulls the ~1.3us tanh ACT_TABLE_LOAD (and the const-bias TENSOR_LOAD)
    # into the input-DMA prologue. A long burst here would delay mm0(0) on
    # the in-order PE queue and add power that invites HAM throttling.
    # ~12 x 128-row dummy matmuls ≈ 1.3us: keeps the PE out of its cold
    # pstate until just before the first input chunk lands (~2.0us after
    # the queue opens) WITHOUT blocking mm0(0) behind them — the PE queue
    # is issue-in-order, so a longer burst would delay the first chunk.
    warm = singles.tile([128, 128], fmm, name="warm", tag="warm")
    nc.vector.memset(warm, 0.0)
    wpsum = psum_mm.tile([128, CHUNK], f32, name="wpsum", tag="mm0")
    for _ in range(12):
        nc.tensor.matmul(wpsum[0:3, 0:128], warm[:, 0:3], warm,
                         start=True, stop=True)
    wact = singles.tile([128, 1], f32, name="wact", tag="wact")
    nc.scalar.activation(wact, warm[:, 0:1], ACTF.Tanh)

    # ---------------- constants / weights in SBUF ----------------
    w_sb = [None] * 4
    b_sb = {}

    def load_w(i):
        w = singles.tile(list(W_d[i].shape), W_d[i].dtype, name=f"w{i}sb",
                         tag=f"w{i}sb")
        nc.sync.dma_start(out=w, in_=W_d[i])
        w_sb[i] = w

    def load_b(i):
        # GpSimd SWDGE queue: parallel to the Sync HWDGE chain.
        b = singles.tile([H, 1], f32, name=f"b{i}sb", tag=f"b{i}sb")
        nc.gpsimd.dma_start(out=b, in_=b_d[i].rearrange("(p one) -> p one", one=1))
        b_sb[i] = b

    # b3 broadcast to all partitions: [128, 3]
    b3bc = singles.tile([128, 3], f32, name="b3bc", tag="b3bc")

    def load_rest():
        for i in (1, 2, 3):
            load_w(i)
            if i < 3:
                load_b(i)
        nc.gpsimd.dma_start(
            out=b3bc,
            in_=bass.AP(tensor=b_d[3].tensor, offset=0, ap=[[0, 128], [1, 3]]),
        )

    # interleaved q/s in order B: partition p holds rows [p*F, (p+1)*F).
    # GpSimd SWDGE queue, deferred into the geometry stream: keeps the
    # Sync HWDGE ring free for the latency-critical early sdd chunk loads.
    iq = singles.tile([128, F, 3], f32, name="iq", tag="iq")
    is_ = singles.tile([128, F, 3], f32, name="is_", tag="is_")

    # MLP output, batch-on-partitions order B: yB3[p, c, f] = y[256p+f, c]
    yB3 = singles.tile([128, 3, F], f32, name="yB3", tag="yB3")
    # consolidated cofactors: C_all[p, 3*j+i, f] = C[j][i] at batch 256p+f
    C_all = singles.tile([128, 9, F], f32, name="C_all", tag="C_all")
    # Kdiag/det, per component: Krd3[p, c, f]
    Krd3 = singles.tile([128, 3, F], f32, name="Krd3", tag="Krd3")
    out_int = singles.tile([128, F, 3], f32, name="out_int", tag="out_int")

    # ---------------- geometry op list (drained between chunks) ----------
    G = {}  # name -> AP

    def gt(name, tag=None):
        t = geo.tile([128, F], f32, name=name, tag=tag or name)
        G[name] = t
        return t

    geo_ops = []

    def deferred(fn):
        geo_ops.append(fn)

    vec = nc.vector

    def op_load_iq():
        nc.gpsimd.dma_start(out=iq, in_=q_d.rearrange("(p f) c -> p f c", p=128))

    def op_load_is():
        nc.gpsimd.dma_start(out=is_, in_=s_d.rearrange("(p f) c -> p f c", p=128))

    deferred(op_load_iq)
    deferred(op_load_is)

    # --- phase 1: t = q^2 and cos(q) (cos gates A which gates cofactors)
    def emit_tcos(c):
        x = iq[:, :, c]

        def op_t():
            t = gt(f"t{c}")
            vec.tensor_mul(t, x, x)
        deferred(op_t)

        def op_cos():
            t = G[f"t{c}"]
            d0, d1, d2, d3, d4, d5 = _CC
            w = gt(f"cw{c}", tag="cw")
            vec.scalar_tensor_tensor(w, t, d4 / d5, t, op0=ALU.add, op1=ALU.mult)
            vec.scalar_tensor_tensor(w, w, d3 / d5, t, op0=ALU.add, op1=ALU.mult)
            vec.scalar_tensor_tensor(w, w, d2 / d5, t, op0=ALU.add, op1=ALU.mult)
            vec.scalar_tensor_tensor(w, w, d1 / d5, t, op0=ALU.add, op1=ALU.mult)
            cq = gt(f"cq{c}")
            vec.tensor_scalar(cq, w, d5, 1.0, op0=ALU.mult, op1=ALU.add)
        deferred(op_cos)

    # --- phase 2: A rows (row index = coordinate, col index = arm c)
    def emit_a(c):
        s0, s1, s2 = is_[:, :, 0], is_[:, :, 1], is_[:, :, 2]

        def op_a():
            cq = G[f"cq{c}"]
            dR = RE - RB
            a0 = gt(f"a0{c}")
            vec.tensor_scalar(a0, cq, -LA * CA[c], dR * CA[c],
                              op0=ALU.mult, op1=ALU.add)
            vec.tensor_add(a0, a0, s0)
            a1 = gt(f"a1{c}")
            vec.tensor_scalar(a1, cq, -LA * SA[c], dR * SA[c],
                              op0=ALU.mult, op1=ALU.add)
            vec.tensor_add(a1, a1, s1)
            a2 = gt(f"a2{c}")
            vec.scalar_tensor_tensor(a2, cq, -LA, s2, op0=ALU.mult, op1=ALU.add)
        deferred(op_a)

    # --- phase 3a: cofactors C[i][j] of entry (i,j); x_i = sum_j C[j][i]*r_j
    COF = [
        ((0, 0), (1, 1), (2, 2), (1, 2), (2, 1)),
        ((0, 1), (1, 2), (2, 0), (1, 0), (2, 2)),
        ((0, 2), (1, 0), (2, 1), (1, 1), (2, 0)),
        ((1, 0), (0, 2), (2, 1), (0, 1), (2, 2)),
        ((1, 1), (0, 0), (2, 2), (0, 2), (2, 0)),
        ((1, 2), (0, 1), (2, 0), (0, 0), (2, 1)),
        ((2, 0), (0, 1), (1, 2), (0, 2), (1, 1)),
        ((2, 1), (0, 2), (1, 0), (0, 0), (1, 2)),
        ((2, 2), (0, 0), (1, 1), (0, 1), (1, 0)),
    ]

    def emit_cof(spec, eng_name):
        (ci, cj), (pi, pj), (pk, pl), (ni, nj), (nk, nl) = spec

        def op():
            eng = getattr(nc, eng_name)
            m1 = gt(f"cm1_{eng_name}_{ci}{cj}", tag=f"cm1_{eng_name}")
            eng.tensor_mul(m1, G[f"a{pi}{pj}"], G[f"a{pk}{pl}"])
            m2 = gt(f"cm2_{eng_name}_{ci}{cj}", tag=f"cm2_{eng_name}")
            eng.tensor_mul(m2, G[f"a{ni}{nj}"], G[f"a{nk}{nl}"])
            eng.tensor_sub(C_all[:, 3 * ci + cj, :], m1, m2)
        deferred(op)

    # --- phase 3b: sin(q) and Kdiag (needed only by Krd, not by cofactors)
    def emit_sin(c):
        x = iq[:, :, c]

        def op_sin():
            t = G[f"t{c}"]
            c1, c3, c5, c7, c9 = _SC
            w = gt(f"sw{c}", tag="sw")
            vec.scalar_tensor_tensor(w, t, c7 / c9, t, op0=ALU.add, op1=ALU.mult)
            vec.scalar_tensor_tensor(w, w, c5 / c9, t, op0=ALU.add, op1=ALU.mult)
            vec.scalar_tensor_tensor(w, w, c3 / c9, t, op0=ALU.add, op1=ALU.mult)
            vec.tensor_scalar(w, w, c9, 1.0, op0=ALU.mult, op1=ALU.add)
            sq = gt(f"sq{c}")
            vec.tensor_mul(sq, w, x)
        deferred(op_sin)

    def emit_k(c):
        s0, s1, s2 = is_[:, :, 0], is_[:, :, 1], is_[:, :, 2]

        def op_k():
            sq, cq = G[f"sq{c}"], G[f"cq{c}"]
            u = gt(f"ku{c}", tag="ku")
            vec.tensor_scalar(u, s0, CA[c], RB - RE, op0=ALU.mult, op1=ALU.add)
            vec.scalar_tensor_tensor(u, s1, SA[c], u, op0=ALU.mult, op1=ALU.add)
            vec.tensor_mul(u, u, sq)
            w = gt(f"kw{c}", tag="kw")
            vec.tensor_mul(w, s2, cq)
            k = gt(f"K{c}")
            vec.tensor_sub(k, u, w)
        deferred(op_k)

    def op_det():
        # det on DVE from the 3 DVE-computed cofactors: no cross-engine
        # wait at the DVE queue head.
        m1 = gt("dm1")
        vec.tensor_mul(m1, G["a00"], C_all[:, 0, :])
        m2 = gt("dm2")
        vec.tensor_mul(m2, G["a01"], C_all[:, 1, :])
        vec.tensor_add(m1, m1, m2)
        vec.tensor_mul(m2, G["a02"], C_all[:, 2, :])
        det = gt("det")
        vec.tensor_add(det, m1, m2)

    def op_rdet():
        rdet = gt("rdet")
        vec.reciprocal(rdet, G["det"])
        for c in range(3):
            vec.tensor_mul(Krd3[:, c, :], G[f"K{c}"], rdet)

    for c in range(3):
        emit_tcos(c)
    for c in range(3):
        emit_a(c)
    # row 0 cofactors (used by det) on DVE first; split the rest so GpSimd
    # finishes adj(A) before the first group combine needs it.
    for idx, spec in enumerate(COF):
        if idx < 3:
            emit_cof(spec, "vector")
        elif idx in (3, 4):
            emit_cof(spec, "vector")
        else:
            emit_cof(spec, "gpsimd")
    deferred(op_det)
    for c in range(3):
        emit_sin(c)
    for c in range(3):
        emit_k(c)
    deferred(op_rdet)

    # ---------------- chunk groups ---------------------------------------
    # (first_chunk, n_chunks) per staging group; the last chunk gets its
    # own mini group so only an 8-column respread+combine+store remains
    # after the final tanh.
    GROUPS = [(4 * g, 4) for g in range(7)] + [(28, 3), (31, 1)]
    CHUNK_GROUP = {}
    for gi, (c0, nch) in enumerate(GROUPS):
        for c in range(c0, c0 + nch):
            CHUNK_GROUP[c] = gi

    def group_fslice(gi):
        c0, nch = GROUPS[gi]
        return 8 * c0, 8 * nch

    # ---------------- per-group combine --------------------------------
    # out[:, :, i] = Krd_i * sum_j C[j][i] * (y_j + b3_j), on the group's
    # f-slice as soon as its yB3 columns land.
    def emit_combine(g):
        s0, nf = group_fslice(g)
        sl = slice(s0, s0 + nf)
        shp = [128, 3, nf]
        m = pool_cmb.tile(shp, f32, name=f"cmb_m{g}", tag="cmb_m")
        t = pool_cmb.tile(shp, f32, name=f"cmb_t{g}", tag="cmb_t")

        def ybr(j):
            return yB3[:, j:j + 1, sl].broadcast_to(shp)

        vec.scalar_tensor_tensor(m, ybr(0), b3bc[:, 0:1], C_all[:, 0:3, sl],
                                 op0=ALU.add, op1=ALU.mult)
        vec.scalar_tensor_tensor(t, ybr(1), b3bc[:, 1:2], C_all[:, 3:6, sl],
                                 op0=ALU.add, op1=ALU.mult)
        vec.tensor_add(m, m, t)
        vec.scalar_tensor_tensor(t, ybr(2), b3bc[:, 2:3], C_all[:, 6:9, sl],
                                 op0=ALU.add, op1=ALU.mult)
        vec.tensor_add(m, m, t)
        vec.tensor_mul(out_int[:, sl, :].transpose([0, 2, 1]), m,
                       Krd3[:, :, sl])
        nc.sync.dma_start(out=outv[:, sl, :], in_=out_int[:, sl, :])

    # ---------------- MLP chunks: 3-stage skewed software pipeline -------
    chunks = _chunks()
    n_chunks = len(chunks)
    n_iters = n_chunks + 2
    per_gap = 8

    PS = {}   # (stage, chunk) -> psum tile
    HT = {}   # (stage, chunk) -> h tile

    def st_dma(ci):
        off, S = chunks[ci]
        sddc = pool_in.tile([4, S], fmm, name=f"sdd_{ci}", tag="sdd")
        if ci == 0:
            # two half-loads: mm0(0) k=0 starts on the first half while the
            # second is still in flight (pipeline ramp-in)
            hS = S // 2
            for k in range(2):
                nc.sync.dma_start(out=sddc[:, hS * k:hS * (k + 1)],
                                  in_=sddT_d[:, off + hS * k:off + hS * (k + 1)])
        else:
            nc.sync.dma_start(out=sddc, in_=sddT_d[:, off:off + S])
        HT[("x", ci)] = sddc

    def st_mm(layer, ci):
        _, S = chunks[ci]
        nS = S // 512
        src = HT[("x", ci)] if layer == 0 else HT[(layer - 1, ci)]
        ps = psum_mm.tile([128, S], f32, name=f"ps{layer}_{ci}",
                          tag=f"mm{layer}")
        for k in range(nS):
            nc.tensor.matmul(ps[:, 512 * k:512 * (k + 1)], w_sb[layer],
                             src[:, 512 * k:512 * (k + 1)],
                             start=True, stop=True)
        PS[(layer, ci)] = ps

    # pipeline-edge tanhs are split in half so the consumer of the first
    # half starts one half-tanh earlier (stream ramp-in / tail ramp-out)
    SPLIT_TANH = {(0, 0), (2, n_chunks - 1)}

    def st_tanh(layer, ci):
        _, S = chunks[ci]
        h = pool_h.tile([128, S], fh, name=f"h{layer}_{ci}", tag="h")
        bias = 0.0 if layer == 0 else b_sb[layer]
        if (layer, ci) in SPLIT_TANH:
            hS = S // 2
            for k in range(2):
                sl = slice(hS * k, hS * (k + 1))
                nc.scalar.activation(h[:, sl], PS[(layer, ci)][:, sl],
                                     ACTF.Tanh, bias=bias)
        else:
            nc.scalar.activation(h, PS[(layer, ci)], ACTF.Tanh, bias=bias)
        HT[(layer, ci)] = h
        del PS[(layer, ci)]

    STG = {}

    def st_l3(ci):
        # Hybrid layer 3. Chunks 0-27 (groups 0-6): W3-stationary [3,512]
        # matmuls — PE-cheap (2 instructions vs 16) — drained by one
        # [3,1024] DVE copy into a (p,vf)-ordered staging tile and
        # respread to yB3 by 3 DMAs per group; this keeps PE under the
        # tanh slot even inside HAM throttle windows. Chunks 28-31: 8
        # "stationary-h3" matmuls per chunk (lhsT = h3 128-column block,
        # rhs = W3 [128,3]) whose [128,3] outputs land directly in
        # batch-on-partitions order (with pi, block m of chunk ci IS yB3
        # f-column 8*ci+m) — pricier on PE per chunk but removes the
        # staging/respread DMA hop from the tail's critical chain.
        h3 = HT[(2, ci)]
        g = CHUNK_GROUP[ci]
        c0, nch = GROUPS[g]
        if ci < 28:
            if g not in STG:
                stg = pool_stg.tile([3, 4 * CHUNK], f32, name=f"stg_{g}",
                                    tag="stg")
                STG[g] = stg.rearrange("c (p v) -> c p v", p=128)
            stg3 = STG[g]
            psl3 = psum_l3.tile([3, CHUNK], f32, name=f"l3_{ci}", tag="l3")
            for k in range(2):
                nc.tensor.matmul(psl3[:, 512 * k:512 * (k + 1)], w_sb[3],
                                 h3[:, 512 * k:512 * (k + 1)],
                                 start=True, stop=True)
            vf0 = 8 * (ci - c0)
            vec.tensor_copy(
                stg3[:, :, vf0:vf0 + 8],
                psl3.rearrange("c (kt p) -> c p kt", p=128),
            )
            if ci == c0 + nch - 1:
                s0, nf = group_fslice(g)
                for c in range(3):
                    nc.sync.dma_start(
                        out=yB3[:, c, s0:s0 + nf],
                        in_=stg3[c:c + 1, :, :],
                    )
                del STG[g]
        else:
            psb = psum_l3.tile([128, 24], f32, name=f"l3_{ci}", tag="l3")
            for m in range(8):
                nc.tensor.matmul(psb[:, 3 * m:3 * (m + 1)],
                                 h3[:, 128 * m:128 * (m + 1)], w_sb[3],
                                 start=True, stop=True)
            vec.tensor_copy(
                yB3[:, :, 8 * ci:8 * ci + 8].transpose([0, 2, 1]),
                psb.rearrange("p (m c) -> p m c", c=3),
            )

    # combine(g) is emitted two chunks into the next group: the group's
    # yB3 copies (same DVE queue) and the geometry it reads are complete
    # by then, so the strict-FIFO DVE queue never head-of-line blocks on
    # it. Group 7 (chunks 28-30) combines right after chunk 30's copy;
    # group 8 (final chunk) right after the last copy.
    combine_at = {4 * g + 5: g for g in range(7)}
    combine_at[30] = 7

    load_w(0)
    st_dma(0)
    st_dma(1)
    load_rest()
    st_mm(0, 0)
    for i in range(n_iters):
        if i + 2 < n_chunks:
            st_dma(i + 2)
        if i + 1 < n_chunks:
            st_mm(0, i + 1)
        if i < n_chunks:
            st_tanh(0, i)
            st_mm(1, i)
        if 0 <= i - 1 < n_chunks:
            st_tanh(1, i - 1)
            st_mm(2, i - 1)
        if 0 <= i - 2 < n_chunks:
            st_tanh(2, i - 2)
            st_l3(i - 2)
            if (i - 2) in combine_at:
                emit_combine(combine_at[i - 2])
        for _ in range(per_gap):
            if geo_ops:
                geo_ops.pop(0)()

    while geo_ops:
        geo_ops.pop(0)()
    emit_combine(8)


def build():
    """Build the per-core Bass program (same program for all 8 cores)."""
    from contextlib import ExitStack

    import concourse.bacc as bacc
    import concourse.tile as tile

    nc = bacc.Bacc(trn_type="TRN2", target_bir_lowering=False, debug=False)
    with tile.TileContext(nc) as tc:
        with ExitStack() as ctx:
            _emit(nc, tc, ctx)
    nc.compile()
    return nc


_NC_CACHE = []


def _shard_inputs(inputs):
    f32 = np.float32
    import ml_dtypes

    fmm = {"f16": np.float16, "f8e4": ml_dtypes.float8_e4m3}[MM_DTYPE]
    fhn = {"f16": np.float16, "f8e4": ml_dtypes.float8_e4m3}[H_DTYPE]
    q = np.ascontiguousarray(np.asarray(inputs["q"], dtype=f32))
    s = np.ascontiguousarray(np.asarray(inputs["s"], dtype=f32))
    sdd = np.asarray(inputs["s_Ddot"], dtype=f32)
    weights = {}
    W0b0 = np.concatenate(
        [np.asarray(inputs["W0"], dtype=f32),
         np.asarray(inputs["b0"], dtype=f32)[None, :]], axis=0)
    weights["W0"] = np.ascontiguousarray(W0b0.astype(fmm))
    for k in ("W1", "W2", "W3"):
        weights[k] = np.ascontiguousarray(np.asarray(inputs[k], dtype=f32).astype(fhn))
    for k in ("b1", "b2", "b3"):
        weights[k] = np.ascontiguousarray(np.asarray(inputs[k], dtype=f32))
    ones = np.ones((1, BC), dtype=fmm)
    in_maps = []
    for c in range(N_CORES):
        sl = slice(c * BC, (c + 1) * BC)
        # pi-permuted transpose: column u = 128f+p holds batch row 256p+f
        X = sdd[sl].reshape(128, F, 3).transpose(2, 1, 0).reshape(3, BC)
        m = {
            "q": q[sl],
            "s": s[sl],
            "sddT": np.ascontiguousarray(
                np.concatenate([X.astype(fmm), ones], axis=0)),
        }
        m.update(weights)
        in_maps.append(m)
    return in_maps


def kernel(**inputs) -> np.ndarray:
    from concourse import bass_utils

    if not _NC_CACHE:
        _NC_CACHE.append(build())
    nc = _NC_CACHE[0]

    in_maps = _shard_inputs(inputs)
    # The axon relay occasionally fails a first execution with
    # NRT_EXEC_UNIT_UNRECOVERABLE; an immediate retry succeeds.
    last_err = None
    for _attempt in range(3):
        try:
            res = bass_utils.run_bass_kernel_spmd(
                nc, in_maps, core_ids=list(range(N_CORES)))
            break
        except Exception as e:  # jax.errors.JaxRuntimeError (transient)
            last_err = e
    else:
        raise last_err
    out = np.concatenate([res.results[c]["out"] for c in range(N_CORES)], axis=0)
    return out.reshape(B_FULL, 3, 1).astype(np.float32)


if __name__ == "__main__":
    nc = build()
    print("built OK")
